# revision 12
# baseline (speedup 1.0000x reference)
"""GCN (2-layer GraphConv + linear classifier) on 8 Trainium2 NeuronCores.

Strategy (graph/data parallel, per the DGL GraphConv norm='both' math):
  - Nodes are sharded 6272/core (N=50000 padded to 50176 = 8*49*128).
  - Each edge is routed to the core that OWNS its dst node.  Edges are
    sorted by dst block (128 nodes), sub-bucketed by src table half
    (so gather indices fit int16), padded to 128-edge tiles, and the
    per-(block,half) tile counts are equalized across cores so a single
    SPMD program serves all 8 cores.
  - The full node-feature table (prescaled by deg_out^-1/2, bf16) is
    replicated in each core's DRAM via AllGather.  Per 128-edge tile,
    a SWDGE dma_gather pulls the 128 source rows into SBUF; the
    scatter-add over dst is a TensorEngine matmul with a one-hot
    selection matrix built on the Vector engine (iota == dst_rel).
    In-degrees fall out of the same matmuls (rhs = ones) in layer 1.
  - Out-degrees come from an analogous prepass over a src-sorted
    stream (no gathers, just selection matmuls against ones).
  - Dense parts per 128-node block: z = aggT.T @ W (PSUM), scale by
    deg_in^-1/2, +bias, relu; layer 2 is computed transposed so the
    final fc reduces to one matmul per block, no transposes needed.
"""

import sys

sys.path.insert(0, "/opt/trn_rl_repo")

from contextlib import ExitStack

import numpy as np
import ml_dtypes

from concourse import bacc, mybir
import concourse.tile as tile
from concourse.masks import make_identity

P = 128
D = 128
NCORES = 8
GCHUNK = 8  # max tiles (128 idxs each) per dma_gather; >1024 idxs is fatal on HW

F32 = mybir.dt.float32
BF16 = mybir.dt.bfloat16
I16 = mybir.dt.int16
BF = ml_dtypes.bfloat16
ALU = mybir.AluOpType
ACTF = mybir.ActivationFunctionType


class Cfg:
    def __init__(self, n_nodes, ncores=NCORES):
        self.n = n_nodes
        self.ncores = ncores
        per = -(-n_nodes // ncores)
        self.shard = -(-per // P) * P
        self.npad = self.shard * ncores
        self.nblk = self.shard // P
        self.half = self.npad // 2
        assert self.half < 32768, "int16 gather indices"


# ---------------------------------------------------------------- host prep


def prep_streams(cfg, src, dst):
    """Bucket/sort/pad the edge lists into per-core device streams.

    Returns dict with per-core packed arrays and the (shared) tile counts.
    """
    nb, nc_ = cfg.nblk, cfg.ncores
    src = src.astype(np.int64)
    dst = dst.astype(np.int64)

    # --- dst-owner streams (aggregation + in-degree) ---
    owner = dst // cfg.shard
    loc = dst % cfg.shard
    blk = loc // P
    rel = (loc % P).astype(np.float32)
    half = (src >= cfg.half).astype(np.int64)
    gidx = (src - half * cfg.half).astype(np.int64)

    key = (owner * nb + blk) * 2 + half
    counts = np.bincount(key, minlength=nc_ * nb * 2).reshape(nc_, nb, 2)
    tiles_bh = (-(-counts // P)).max(axis=0)  # [nb, 2]
    T = int(tiles_bh.sum())

    order = np.argsort(key, kind="stable")
    csum = np.concatenate([[0], np.cumsum(counts.reshape(-1))]).astype(np.int64)
    toff = np.concatenate([[0], np.cumsum(tiles_bh.reshape(-1))]).astype(np.int64)

    sidx_s = np.zeros((nc_, T * P), np.int16)
    drel_s = np.full((nc_, T * P), -1.0, np.float32)
    for c in range(nc_):
        for b in range(nb):
            for h in range(2):
                gi = (c * nb + b) * 2 + h
                eids = order[csum[gi] : csum[gi + 1]]
                n = len(eids)
                off = int(toff[b * 2 + h]) * P
                sidx_s[c, off : off + n] = gidx[eids]
                drel_s[c, off : off + n] = rel[eids]

    sidx_p = np.tile(
        sidx_s.reshape(nc_, T * 8, 16).transpose(0, 2, 1), (1, 8, 1)
    )  # [nc, 128, T*8]
    drel_p = drel_s.reshape(nc_, T, P).transpose(0, 2, 1).copy()  # [nc, 128, T] f32

    # --- src-owner stream (out-degree prepass; no gathers) ---
    sowner = src // cfg.shard
    sloc = src % cfg.shard
    sblk = sloc // P
    srel_v = (sloc % P).astype(np.float32)
    skey = sowner * nb + sblk
    scounts = np.bincount(skey, minlength=nc_ * nb).reshape(nc_, nb)
    stiles = (-(-scounts // P)).max(axis=0)  # [nb]
    ST = int(stiles.sum())

    sorder = np.argsort(skey, kind="stable")
    scsum = np.concatenate([[0], np.cumsum(scounts.reshape(-1))]).astype(np.int64)
    stoff = np.concatenate([[0], np.cumsum(stiles)]).astype(np.int64)

    srel_s = np.full((nc_, max(ST, 1) * P), -1.0, np.float32)
    for c in range(nc_):
        for b in range(nb):
            gi = c * nb + b
            eids = sorder[scsum[gi] : scsum[gi + 1]]
            n = len(eids)
            off = int(stoff[b]) * P
            srel_s[c, off : off + n] = srel_v[eids]
    srel_p = (
        srel_s.reshape(nc_, max(ST, 1), P).transpose(0, 2, 1).copy()
    )  # [nc, 128, ST] f32

    return dict(
        tiles_bh=tiles_bh,
        stiles=stiles,
        T=T,
        ST=ST,
        sidx=sidx_p,
        drel=drel_p,
        srel=srel_p,
    )


# ---------------------------------------------------------------- builder


def build(cfg, tiles_bh, stiles):
    nb = cfg.nblk
    T = int(tiles_bh.sum())
    ST = int(stiles.sum())
    STm = max(ST, 1)

    nc = bacc.Bacc("TRN2", target_bir_lowering=False, debug=False)

    feat_ext = nc.dram_tensor("feat", [cfg.shard, D], F32, kind="ExternalInput")
    sidx_ext = nc.dram_tensor("sidx", [P, T * 8], I16, kind="ExternalInput")
    drel_ext = nc.dram_tensor("drel", [P, T], F32, kind="ExternalInput")
    srel_ext = nc.dram_tensor("srel", [P, STm], F32, kind="ExternalInput")
    w1_ext = nc.dram_tensor("w1", [D, D], F32, kind="ExternalInput")
    w2_ext = nc.dram_tensor("w2", [D, D], F32, kind="ExternalInput")
    b1_ext = nc.dram_tensor("b1c", [D, 1], F32, kind="ExternalInput")
    b2_ext = nc.dram_tensor("b2c", [D, 1], F32, kind="ExternalInput")
    fcw_ext = nc.dram_tensor("fcw", [D, 1], F32, kind="ExternalInput")
    cst_ext = nc.dram_tensor("cst", [1, 1], F32, kind="ExternalInput")  # fc_b-thres
    out_ext = nc.dram_tensor("out", [1, cfg.shard], F32, kind="ExternalOutput")

    groups = [list(range(cfg.ncores))]

    # DRAM tables (raw internal tensors; DRAM tile-pools crash walrus codegen)
    tbl1_shard = nc.dram_tensor("tbl1_shard", [cfg.shard, D], BF16)
    tbl2_shard = nc.dram_tensor("tbl2_shard", [cfg.shard, D], BF16)
    tbl1 = nc.dram_tensor("tbl1", [cfg.npad, D], BF16, addr_space="Shared")
    tbl2 = nc.dram_tensor("tbl2", [cfg.npad, D], BF16, addr_space="Shared")

    with tile.TileContext(nc) as tc, ExitStack() as stk:

        # ---- constants ----
        cpool = stk.enter_context(tc.tile_pool(name="consts", bufs=1))
        iota_i = cpool.tile([P, P], I16)
        nc.gpsimd.iota(iota_i[:], pattern=[[1, P]], base=0, channel_multiplier=0)
        iota_bf = cpool.tile([P, P], BF16)
        nc.vector.tensor_copy(iota_bf[:], iota_i[:])
        ones_col = cpool.tile([P, 1], BF16)
        nc.vector.memset(ones_col[:], 1.0)
        ident = cpool.tile([P, P], F32)
        make_identity(nc, ident[:])

        w1_bf = cpool.tile([D, D], BF16)
        w2_bf = cpool.tile([D, D], BF16)
        for ext, bft in ((w1_ext, w1_bf), (w2_ext, w2_bf)):
            wf = cpool.tile([D, D], F32, tag="wtmp")
            nc.sync.dma_start(wf[:], ext[:])
            nc.vector.tensor_copy(bft[:], wf[:])
        b1_col = cpool.tile([D, 1], F32)
        nc.sync.dma_start(b1_col[:], b1_ext[:])
        b2_col = cpool.tile([D, 1], F32)
        nc.sync.dma_start(b2_col[:], b2_ext[:])
        fcw_f = cpool.tile([D, 1], F32)
        nc.sync.dma_start(fcw_f[:], fcw_ext[:])
        fcw_bf = cpool.tile([D, 1], BF16)
        nc.vector.tensor_copy(fcw_bf[:], fcw_f[:])
        cst = cpool.tile([1, 1], F32)
        nc.sync.dma_start(cst[:], cst_ext[:])

        # b1 broadcast [P,P]: row j = b1[j] for every partition
        b1b = cpool.tile([P, P], F32)
        with tc.tile_pool(name="pinit", bufs=1, space="PSUM") as pinit:
            b1bp = pinit.tile([P, P], F32)
            nc.tensor.transpose(
                out=b1bp[:], in_=b1_col[:].to_broadcast([P, P]), identity=ident[:]
            )
            nc.vector.tensor_copy(b1b[:], b1bp[:])

        # per-node normalizers (per block columns)
        dout_all = cpool.tile([P, nb], F32)
        din_all = cpool.tile([P, nb], F32)
        din_bc = cpool.tile([P, cfg.shard], BF16)

        # edge streams
        sidx_sb = cpool.tile([P, T * 8], I16)
        nc.sync.dma_start(sidx_sb[:], sidx_ext[:])
        drel_sb = cpool.tile([P, T], F32)
        nc.sync.dma_start(drel_sb[:], drel_ext[:])
        srel_sb = cpool.tile([P, STm], F32)
        nc.sync.dma_start(srel_sb[:], srel_ext[:])

        spool = stk.enter_context(tc.tile_pool(name="sel", bufs=6))
        wpool = stk.enter_context(tc.tile_pool(name="work", bufs=4))
        iopool = stk.enter_context(tc.tile_pool(name="io", bufs=4))
        mpool = stk.enter_context(tc.tile_pool(name="msg", bufs=4))
        ppool = stk.enter_context(tc.tile_pool(name="pagg", bufs=2, space="PSUM"))
        ppool2 = stk.enter_context(tc.tile_pool(name="pz", bufs=2, space="PSUM"))
        ppooldb = stk.enter_context(tc.tile_pool(name="pdb", bufs=1, space="PSUM"))
        ppool3 = stk.enter_context(tc.tile_pool(name="psmall", bufs=2, space="PSUM"))

        # ---- out-degree prepass (src-sorted stream) ----
        scol = 0
        for b in range(nb):
            nt = int(stiles[b])
            if nt == 0:
                nc.vector.memset(dout_all[:, b : b + 1], 1.0)
                continue
            degp = ppool3.tile([P, 1], F32, tag="deg")
            for t in range(nt):
                S = spool.tile([P, P], BF16, tag="S")
                nc.vector.tensor_scalar(
                    out=S[:],
                    in0=iota_bf[:],
                    scalar1=srel_sb[:, scol + t : scol + t + 1],
                    scalar2=None,
                    op0=ALU.is_equal,
                )
                nc.tensor.matmul(
                    degp[:], lhsT=S[:], rhs=ones_col[:], start=(t == 0), stop=(t == nt - 1)
                )
            dmx = wpool.tile([P, 1], F32, tag="dmx")
            nc.vector.tensor_scalar(
                out=dmx[:], in0=degp[:], scalar1=1.0, scalar2=None, op0=ALU.max
            )
            drc = wpool.tile([P, 1], F32, tag="drc")
            nc.vector.reciprocal(drc[:], dmx[:])
            nc.scalar.activation(dout_all[:, b : b + 1], drc[:], ACTF.Sqrt)
            scol += nt

        # ---- layer-1 table: feat * dout, bf16, allgather ----
        for b in range(nb):
            ft = iopool.tile([P, D], F32, tag="ft")
            nc.sync.dma_start(ft[:], feat_ext[b * P : (b + 1) * P, :])
            tt = iopool.tile([P, D], BF16, tag="tt")
            nc.vector.tensor_scalar(
                out=tt[:],
                in0=ft[:],
                scalar1=dout_all[:, b : b + 1],
                scalar2=None,
                op0=ALU.mult,
            )
            nc.sync.dma_start(tbl1_shard[b * P : (b + 1) * P, :], tt[:])
        nc.gpsimd.collective_compute(
            "AllGather",
            ALU.bypass,
            replica_groups=groups,
            ins=[tbl1_shard[:]],
            outs=[tbl1[:]],
        )

        # ---- layers ----
        def layer(L, tbl_full):
            col = 0
            for b in range(nb):
                ntl, nth = int(tiles_bh[b, 0]), int(tiles_bh[b, 1])
                nt = ntl + nth
                if nt == 0:
                    agg_sb = wpool.tile([P, P], BF16, tag="agg")
                    nc.vector.memset(agg_sb[:], 0.0)
                    if L == 1:
                        nc.vector.memset(din_all[:, b : b + 1], 1.0)
                        nc.vector.memset(din_bc[:, b * P : (b + 1) * P], 1.0)
                else:
                    mt = mpool.tile([P, nt, D], BF16, tag="mt")
                    # HW limit: dma_gather dies above 1024 idxs/instruction
                    for lo_t, n_t, tb in ((0, ntl, 0), (ntl, nth, 1)):
                        for c0 in range(0, n_t, GCHUNK):
                            cn = min(GCHUNK, n_t - c0)
                            a = lo_t + c0
                            nc.gpsimd.dma_gather(
                                mt[:, a : a + cn, :],
                                tbl_full[tb * cfg.half : (tb + 1) * cfg.half, :],
                                sidx_sb[:, (col + a) * 8 : (col + a + cn) * 8],
                                cn * P,
                                cn * P,
                                D,
                            )
                    aggp = ppool.tile([P, P], F32, tag="aggp")
                    if L == 1:
                        degp = ppool3.tile([P, 1], F32, tag="deg")
                    for t in range(nt):
                        S = spool.tile([P, P], BF16, tag="S")
                        nc.vector.tensor_scalar(
                            out=S[:],
                            in0=iota_bf[:],
                            scalar1=drel_sb[:, col + t : col + t + 1],
                            scalar2=None,
                            op0=ALU.is_equal,
                        )
                        nc.tensor.matmul(
                            aggp[:],
                            lhsT=mt[:, t, :],
                            rhs=S[:],
                            start=(t == 0),
                            stop=(t == nt - 1),
                        )
                        if L == 1:
                            nc.tensor.matmul(
                                degp[:],
                                lhsT=S[:],
                                rhs=ones_col[:],
                                start=(t == 0),
                                stop=(t == nt - 1),
                            )
                    if L == 1:
                        dmx = wpool.tile([P, 1], F32, tag="dmx")
                        nc.vector.tensor_scalar(
                            out=dmx[:], in0=degp[:], scalar1=1.0, scalar2=None, op0=ALU.max
                        )
                        drc = wpool.tile([P, 1], F32, tag="drc")
                        nc.vector.reciprocal(drc[:], dmx[:])
                        nc.scalar.activation(din_all[:, b : b + 1], drc[:], ACTF.Sqrt)
                        dbp = ppooldb.tile([P, P], F32, tag="dbp")
                        nc.tensor.transpose(
                            out=dbp[:],
                            in_=din_all[:, b : b + 1].to_broadcast([P, P]),
                            identity=ident[:],
                        )
                        nc.vector.tensor_copy(din_bc[:, b * P : (b + 1) * P], dbp[:])
                    agg_sb = wpool.tile([P, P], BF16, tag="agg")
                    if L == 1:
                        nc.vector.tensor_copy(agg_sb[:], aggp[:])
                    else:
                        nc.vector.tensor_tensor(
                            out=agg_sb[:],
                            in0=aggp[:],
                            in1=din_bc[:, b * P : (b + 1) * P],
                            op=ALU.mult,
                        )

                if L == 1:
                    # z[node, fout] = aggT.T @ W1 ; h1 = relu(din*z + b1) ; tbl2 = h1*dout
                    z = ppool2.tile([P, P], F32, tag="z")
                    nc.tensor.matmul(z[:], lhsT=agg_sb[:], rhs=w1_bf[:], start=True, stop=True)
                    t1 = wpool.tile([P, P], F32, tag="t1")
                    nc.vector.tensor_scalar(
                        out=t1[:],
                        in0=z[:],
                        scalar1=din_all[:, b : b + 1],
                        scalar2=None,
                        op0=ALU.mult,
                    )
                    t2 = wpool.tile([P, P], F32, tag="t2")
                    nc.vector.tensor_tensor(out=t2[:], in0=t1[:], in1=b1b[:], op=ALU.add)
                    h1 = wpool.tile([P, P], F32, tag="h1")
                    nc.scalar.activation(h1[:], t2[:], ACTF.Relu)
                    tt2 = wpool.tile([P, P], BF16, tag="tt2")
                    nc.vector.tensor_scalar(
                        out=tt2[:],
                        in0=h1[:],
                        scalar1=dout_all[:, b : b + 1],
                        scalar2=None,
                        op0=ALU.mult,
                    )
                    nc.sync.dma_start(tbl2_shard[b * P : (b + 1) * P, :], tt2[:])
                else:
                    # z2T[fout, node] = W2.T @ (din*agg) ; h2T = relu(z2T + b2)
                    z2 = ppool2.tile([P, P], F32, tag="z")
                    nc.tensor.matmul(z2[:], lhsT=w2_bf[:], rhs=agg_sb[:], start=True, stop=True)
                    h2 = wpool.tile([P, P], BF16, tag="h2")
                    nc.scalar.activation(
                        h2[:], z2[:], ACTF.Relu, bias=b2_col[:, 0:1], scale=1.0
                    )
                    lgp = ppool3.tile([1, P], F32, tag="deg")
                    nc.tensor.matmul(lgp[:], lhsT=fcw_bf[:], rhs=h2[:], start=True, stop=True)
                    lg = wpool.tile([1, P], F32, tag="lgs")
                    nc.vector.tensor_scalar(
                        out=lg[:], in0=lgp[:], scalar1=cst[0:1, 0:1], scalar2=None, op0=ALU.add
                    )
                    nc.sync.dma_start(out_ext[0:1, b * P : (b + 1) * P], lg[:])
                col += nt

        layer(1, tbl1)
        nc.gpsimd.collective_compute(
            "AllGather",
            ALU.bypass,
            replica_groups=groups,
            ins=[tbl2_shard[:]],
            outs=[tbl2[:]],
        )
        layer(2, tbl2)

    nc.compile()
    return nc


# ---------------------------------------------------------------- entry


def make_in_maps(cfg, streams, features, W1, b1, W2, b2, fc_w, fc_b, cl_thres):
    n, sh = cfg.n, cfg.shard
    featp = np.zeros((cfg.npad, D), np.float32)
    featp[:n] = np.asarray(features, np.float32)
    cstv = np.asarray(fc_b, np.float32).reshape(-1)[0] - np.float32(
        np.asarray(cl_thres).reshape(-1)[0]
    )
    in_maps = []
    for c in range(cfg.ncores):
        in_maps.append(
            {
                "feat": featp[c * sh : (c + 1) * sh].copy(),
                "sidx": streams["sidx"][c].copy(),
                "drel": streams["drel"][c].copy(),
                "srel": streams["srel"][c].copy(),
                "w1": np.asarray(W1, np.float32),
                "w2": np.asarray(W2, np.float32),
                "b1c": np.asarray(b1, np.float32).reshape(D, 1),
                "b2c": np.asarray(b2, np.float32).reshape(D, 1),
                "fcw": np.asarray(fc_w, np.float32).reshape(D, 1),
                "cst": np.asarray(cstv, np.float32).reshape(1, 1),
            }
        )
    return in_maps


def _install_ntff_hook():
    """Recreate the antenv.axon_hooks module the boot shim degrades without,
    and register the ctypes NTFF profile hook so trace=True works."""
    import types

    if "antenv.axon_hooks" in sys.modules:
        return
    import antenv
    from trn_agent_boot.trn_boot import _ntff_profile_via_ctypes

    mod = types.ModuleType("antenv.axon_hooks")
    state = {"h": None}
    mod.set_axon_ntff_profile_hook = lambda h: state.__setitem__("h", h)
    mod.get_axon_ntff_profile_hook = lambda: state["h"]
    sys.modules["antenv.axon_hooks"] = mod
    antenv.axon_hooks = mod
    mod.set_axon_ntff_profile_hook(
        _ntff_profile_via_ctypes("/opt/axon/libaxon_pjrt.so")
    )


def kernel(features, src, dst, W1, b1, W2, b2, fc_w, fc_b, cl_thres, _trace=False):
    from concourse.bass_utils import run_bass_kernel_spmd

    if _trace:
        try:
            _install_ntff_hook()
        except Exception as e:
            print(f"ntff hook install failed ({e}); running without trace")
            _trace = False

    import time as _time

    _t0 = _time.time()
    features = np.asarray(features)
    cfg = Cfg(features.shape[0])
    streams = prep_streams(cfg, np.asarray(src), np.asarray(dst))
    print(f"[kernel] prep done {_time.time()-_t0:.1f}s", flush=True)
    nc = build(cfg, streams["tiles_bh"], streams["stiles"])
    print(f"[kernel] build done {_time.time()-_t0:.1f}s", flush=True)
    in_maps = make_in_maps(
        cfg, streams, features, W1, b1, W2, b2, fc_w, fc_b, cl_thres
    )
    res = run_bass_kernel_spmd(
        nc, in_maps, list(range(cfg.ncores)), trace=_trace
    )
    print(f"[kernel] run done {_time.time()-_t0:.1f}s", flush=True)
    out = np.concatenate([res.results[c]["out"][0] for c in range(cfg.ncores)])
    kernel.last_exec_time_ns = res.exec_time_ns
    return out[: cfg.n].reshape(cfg.n, 1).astype(np.float32)
